# revision 13
# baseline (speedup 1.0000x reference)
"""TTFS (time-to-first-spike) encoder kernel for Trainium2, 8 NeuronCores.

Math: the reference runs, per element, the fp32 recurrence
    mem_k = fl(fl(mem_{k-1} * d) + fl(cur * (1-d))),   d = fl(exp(-0.5f))
and emits a one-hot over time at the first k with mem_k >= 1.0 (later spikes
are masked).  mem_k is monotone in cur (a composition of monotone rounded
ops), so "first crossing at step k" is exactly a threshold test on cur:
    spike at out[t] iff THETA[t+1] <= cur < THETA[t]      (THETA[0] = +inf)
where THETA[k] = min fp32 c with mem_k(c) >= 1.0, found by binary search over
the fp32 bit space against a bit-exact host simulation of the recurrence.
The fp32 recurrence converges by step 32: THETA[32] == THETA[33] == ... ==
THETA[64], so out[:, t, :] == 0 for all t >= 32 for EVERY input; the device
only computes/writes slabs t = 0..31 and the host zero-fills the rest.

Device work per core (batch-sharded 2048/8 = 256 rows, laid out as
[128 partitions x 2048] with the two 128-row halves side by side in the
free dim):
    cur   = x * sensitivity        (PE broadcasts sensitivity to 128 rows)
    s_k   = [cur >= THETA[k]]  as either
              clean cmp:  Vector tensor_scalar is_ge -> {0,1}
                          or Scalar Sign(Relu(cur - pred(THETA[k])))
              dirty cmp:  Scalar r = Relu(cur - pred(THETA[k]))
                          (r > 0 <=> cur >= THETA[k]; exact: the sign of a
                          rounded difference is the true sign, and the
                          smallest positive gap ~1.2e-7 survives bf16)
    out[t] = s_{t+1} - s_t  on Vector, as scalar_tensor_tensor ops that clean
    a dirty operand inline:  ([r_{t+1}>0]) subtract s_t   (dirty minuend)
                             ([r_t>0]) is_lt s_{t+1}      (dirty subtrahend)
Dirty cmps are cheap (one Scalar op) and alternate with clean cmps so no
subtract sees two dirty inputs.  Work is split across Vector and Scalar to
balance their spans.  Output slabs are bf16 holding exact 0.0/1.0; the host
casts to fp32.
"""

import numpy as np

from concourse import bacc, mybir
from concourse import tile
from concourse.bass_utils import run_bass_kernel_spmd

# THETA[k], k = 1..32, as fp32 bit patterns (see module docstring).
_THETA_BITS = [
    0x4022A7D7, 0x3FCA7E37, 0x3FA4C386, 0x3F9408C5,
    0x3F8B724C, 0x3F86B4E7, 0x3F83FC52, 0x3F82635E,
    0x3F81701C, 0x3F80DE49, 0x3F808677, 0x3F80516D,
    0x3F803157, 0x3F801DE8, 0x3F801222, 0x3F800B00,
    0x3F8006AB, 0x3F80040B, 0x3F800274, 0x3F80017D,
    0x3F8000E7, 0x3F80008C, 0x3F800055, 0x3F800034,
    0x3F80001F, 0x3F800013, 0x3F80000C, 0x3F800007,
    0x3F800005, 0x3F800002, 0x3F800002, 0x3F800001,
]
THETAS = np.array(_THETA_BITS, dtype=np.uint32).view(np.float32)
# pred(THETA[k]): one ulp below (all values are positive normals)
PHIS = (np.array(_THETA_BITS, dtype=np.uint32) - 1).view(np.float32)

N_CORES = 8
B, T, N = 2048, 64, 1024
BS = B // N_CORES          # 256 batch rows per core
P = 128                    # SBUF partitions
W = 2 * N                  # fused free width (two 128-row halves)
TS = 32                    # device-computed time slabs (rest are zero)
TC = 2                     # timesteps per DMA chunk

F32 = mybir.dt.float32
BF16 = mybir.dt.bfloat16
FP8 = mybir.dt.float8e4

# Comparison t is "dirty" (Scalar Relu only) for odd t <= 29; these alternate
# with clean cmps so every subtract has at least one clean operand.  Of the
# clean cmps, ACT_SIGN_SET run on Scalar as 2-op Sign(Relu(.)) to balance
# engine spans; the rest are Vector tensor_scalar is_ge.
DIRTY_SET = frozenset(t for t in range(1, 30, 2))
ACT_SIGN_SET = frozenset({4, 10, 16, 22, 28, 31})
SCALE_HI = 2.0 ** 60   # exact power-of-two prescale for dirty cmps


def _build():
    nc = bacc.Bacc("TRN2", target_bir_lowering=False, debug=False)
    x_d = nc.dram_tensor("x", [BS, N], F32, kind="ExternalInput")
    sens_d = nc.dram_tensor("sens", [1, N], F32, kind="ExternalInput")
    out_d = nc.dram_tensor("out", [BS, TS, N], BF16, kind="ExternalOutput")

    # b = h*128 + p  ->  partition p, free-dim half h
    x_v = x_d.rearrange("(h p) n -> p h n", h=2)
    out_v = out_d.rearrange("(h p) t n -> p t h n", h=2)

    with tile.TileContext(nc) as tc:
        with (
            tc.tile_pool(name="const", bufs=1) as cpool,
            tc.tile_pool(name="psum", bufs=2, space="PSUM") as ppool,
            tc.tile_pool(name="s", bufs=6) as spool,
            tc.tile_pool(name="r", bufs=4) as rpool,
            tc.tile_pool(name="slab", bufs=4) as slabpool,
        ):
            ones = cpool.tile([1, P], F32)
            nc.vector.memset(ones[:], 1.0)
            sens_sb = cpool.tile([1, N], F32)
            nc.sync.dma_start(sens_sb[:], sens_d[:, :])
            sens_bc = cpool.tile([P, W], F32)
            for half in range(2):
                ps = ppool.tile([P, 512], F32)
                nc.tensor.matmul(
                    ps[:], ones[:], sens_sb[:, half * 512:(half + 1) * 512],
                    start=True, stop=True,
                )
                for h in range(2):
                    nc.vector.tensor_copy(
                        sens_bc[:, h * N + half * 512:h * N + (half + 1) * 512],
                        ps[:],
                    )

            act_bias, act_bias_hi = {}, {}
            for t in sorted(ACT_SIGN_SET):
                bt = cpool.tile([P, 1], F32, tag=f"bias{t}")
                nc.gpsimd.memset(bt[:], float(-PHIS[t]))
                act_bias[t] = bt
            for t in sorted(DIRTY_SET):
                bt = cpool.tile([P, 1], F32, tag=f"biash{t}")
                nc.gpsimd.memset(bt[:], float(np.float32(-PHIS[t])
                                              * np.float32(SCALE_HI)))
                act_bias_hi[t] = bt

            xt = cpool.tile([P, W], F32)
            nc.sync.dma_start(xt[:], x_v[:, :])
            cur = cpool.tile([P, W], F32)
            nc.vector.tensor_tensor(cur[:], xt[:], sens_bc[:],
                                    mybir.AluOpType.mult)

            s_prev, prev_dirty = None, False
            for tchunk in range(TS // TC):
                slab = slabpool.tile([P, TC * W], BF16, tag="slab")
                for tt in range(TC):
                    t = tchunk * TC + tt
                    dst = slab[:, tt * W:(tt + 1) * W]
                    if t > 0 and THETAS[t] == THETAS[t - 1]:
                        # empty band: s_{t+1} == s_t, slab is identically 0
                        nc.gpsimd.memset(dst, 0.0)
                        continue
                    s = spool.tile([P, W], BF16, tag="s")
                    dirty = t in DIRTY_SET
                    if dirty:
                        # r' = Relu(2^60*(cur - phi)): 0 iff cur < THETA[t],
                        # else >= 2^60*ulp ~ 1.4e11 (>= 1 after bf16)
                        nc.scalar.activation(
                            s[:], cur[:], mybir.ActivationFunctionType.Relu,
                            bias=act_bias_hi[t][:], scale=float(SCALE_HI),
                        )
                    elif t in ACT_SIGN_SET:
                        r = rpool.tile([P, W], BF16, tag="r")
                        nc.scalar.activation(
                            r[:], cur[:], mybir.ActivationFunctionType.Relu,
                            bias=act_bias[t][:], scale=1.0,
                        )
                        nc.scalar.activation(
                            s[:], r[:], mybir.ActivationFunctionType.Sign,
                        )
                    else:
                        nc.vector.tensor_scalar(
                            s[:], cur[:], float(THETAS[t]), None,
                            mybir.AluOpType.is_ge,
                        )
                    if t == 0:
                        nc.vector.tensor_copy(dst, s[:])
                    elif dirty:
                        # out = [r_t > 0] - s_{t-1}   (clean subtrahend)
                        nc.vector.scalar_tensor_tensor(
                            dst, s[:], 0.0, s_prev[:],
                            mybir.AluOpType.is_gt, mybir.AluOpType.subtract,
                        )
                    elif prev_dirty:
                        # out = [r'_{t-1} < s_t]: r'=0 -> s_t; r'>=1.4e11 ->
                        # 0 (and then s_t==s_{t+1}==1), both match s_{t+1}-s_t
                        nc.vector.tensor_tensor(dst, s_prev[:], s[:],
                                                mybir.AluOpType.is_lt)
                    else:
                        nc.vector.tensor_tensor(dst, s[:], s_prev[:],
                                                mybir.AluOpType.subtract)
                    s_prev, prev_dirty = s, dirty
                for h in range(2):
                    src = slab[:].rearrange("p (t h n) -> p t h n",
                                            t=TC, h=2, n=N)[:, :, h, :]
                    nc.sync.dma_start(
                        out_d[h * P:(h + 1) * P,
                              tchunk * TC:(tchunk + 1) * TC, :],
                        src,
                    )
    nc.compile()
    return nc


_NC = None


def _get_nc():
    global _NC
    if _NC is None:
        _NC = _build()
    return _NC


def kernel(x, sensitivity):
    x = np.ascontiguousarray(np.asarray(x, dtype=np.float32))
    sens = np.ascontiguousarray(
        np.asarray(sensitivity, dtype=np.float32)
    ).reshape(1, N)
    nc = _get_nc()
    in_maps = [
        {"x": x[c * BS:(c + 1) * BS], "sens": sens} for c in range(N_CORES)
    ]
    res = run_bass_kernel_spmd(nc, in_maps, list(range(N_CORES)))
    dev = np.concatenate(
        [np.asarray(r["out"]) for r in res.results], axis=0
    )  # [B, TS, N] fp8, exact 0/1
    out = np.zeros((B, T, N), dtype=np.float32)
    out[:, :TS, :] = dev.astype(np.float32)
    return out


# revision 15
# speedup vs baseline: 1.0202x; 1.0202x over previous
"""TTFS (time-to-first-spike) encoder kernel for Trainium2, 8 NeuronCores.

Math: the reference runs, per element, the fp32 recurrence
    mem_k = fl(fl(mem_{k-1} * d) + fl(cur * (1-d))),   d = fl(exp(-0.5f))
and emits a one-hot over time at the first k with mem_k >= 1.0 (later spikes
are masked).  mem_k is monotone in cur (a composition of monotone rounded
ops), so "first crossing at step k" is exactly a threshold test on cur:
    spike at out[t] iff THETA[t+1] <= cur < THETA[t]      (THETA[0] = +inf)
where THETA[k] = min fp32 c with mem_k(c) >= 1.0, found by binary search over
the fp32 bit space against a bit-exact host simulation of the recurrence.
The fp32 recurrence converges by step 32: THETA[32] == THETA[33] == ... ==
THETA[64], so out[:, t, :] == 0 for all t >= 32 for EVERY input; the device
only computes/writes slabs t = 0..31 and the host zero-fills the rest.

Device work per core (batch-sharded 2048/8 = 256 rows, laid out as
[128 partitions x 2048] with the two 128-row halves side by side in the
free dim):
    cur   = x * sensitivity        (PE broadcasts sensitivity to 128 rows)
    s_k   = [cur >= THETA[k]]  as either
              clean cmp:  Vector tensor_scalar is_ge -> {0,1}
                          or Scalar Sign(Relu(cur - pred(THETA[k])))
              dirty cmp:  Scalar r = Relu(cur - pred(THETA[k]))
                          (r > 0 <=> cur >= THETA[k]; exact: the sign of a
                          rounded difference is the true sign, and the
                          smallest positive gap ~1.2e-7 survives bf16)
    out[t] = s_{t+1} - s_t  on Vector, as scalar_tensor_tensor ops that clean
    a dirty operand inline:  ([r_{t+1}>0]) subtract s_t   (dirty minuend)
                             ([r_t>0]) is_lt s_{t+1}      (dirty subtrahend)
Dirty cmps are cheap (one Scalar op) and alternate with clean cmps so no
subtract sees two dirty inputs.  Work is split across Vector and Scalar to
balance their spans.  Output slabs are bf16 holding exact 0.0/1.0; the host
casts to fp32.
"""

import numpy as np

from concourse import bacc, mybir
from concourse import tile
from concourse.bass_utils import run_bass_kernel_spmd

# THETA[k], k = 1..32, as fp32 bit patterns (see module docstring).
_THETA_BITS = [
    0x4022A7D7, 0x3FCA7E37, 0x3FA4C386, 0x3F9408C5,
    0x3F8B724C, 0x3F86B4E7, 0x3F83FC52, 0x3F82635E,
    0x3F81701C, 0x3F80DE49, 0x3F808677, 0x3F80516D,
    0x3F803157, 0x3F801DE8, 0x3F801222, 0x3F800B00,
    0x3F8006AB, 0x3F80040B, 0x3F800274, 0x3F80017D,
    0x3F8000E7, 0x3F80008C, 0x3F800055, 0x3F800034,
    0x3F80001F, 0x3F800013, 0x3F80000C, 0x3F800007,
    0x3F800005, 0x3F800002, 0x3F800002, 0x3F800001,
]
THETAS = np.array(_THETA_BITS, dtype=np.uint32).view(np.float32)
# pred(THETA[k]): one ulp below (all values are positive normals)
PHIS = (np.array(_THETA_BITS, dtype=np.uint32) - 1).view(np.float32)

N_CORES = 8
B, T, N = 2048, 64, 1024
BS = B // N_CORES          # 256 batch rows per core
P = 128                    # SBUF partitions
W = 2 * N                  # fused free width (two 128-row halves)
TS = 32                    # device-computed time slabs (rest are zero)
TC = 2                     # timesteps per DMA chunk

F32 = mybir.dt.float32
BF16 = mybir.dt.bfloat16
FP8 = mybir.dt.float8e4

# Comparison t is "dirty" (Scalar Relu only) for odd t <= 29; these alternate
# with clean cmps so every subtract has at least one clean operand.  Of the
# clean cmps, ACT_SIGN_SET run on Scalar as 2-op Sign(Relu(.)) to balance
# engine spans; the rest are Vector tensor_scalar is_ge.
# t=0 cmp is only ever a minuend-source for slab 0 (a direct copy) and the
# subtrahend of slab 1 -> up-scaled dirty works; t=31 cmp is only the minuend
# of slab 31 -> down-scaled dirty works.  All other dirty placements force a
# slow 1x scalar_tensor_tensor, so they are disabled.
DIRTY_UP_SET = frozenset({0})
DIRTY_DOWN_SET = frozenset({31})
ACT_SIGN_SET = frozenset({1, 3, 5, 7, 9, 11, 13, 15, 17, 19, 21, 23, 26})
SCALE_HI = 2.0 ** 60    # exact pow2 prescale: dirty-up values {0} u [1.4e11,..]
SCALE_LO = 2.0 ** -60   # exact pow2 prescale: dirty-down values {0} u (..,7e-18]


def _build():
    nc = bacc.Bacc("TRN2", target_bir_lowering=False, debug=False)
    x_d = nc.dram_tensor("x", [BS, N], F32, kind="ExternalInput")
    sens_d = nc.dram_tensor("sens", [P, W], F32, kind="ExternalInput")
    out_d = nc.dram_tensor("out", [BS, TS, N], BF16, kind="ExternalOutput")

    # b = h*128 + p  ->  partition p, free-dim half h
    x_v = x_d.rearrange("(h p) n -> p h n", h=2)
    out_v = out_d.rearrange("(h p) t n -> p t h n", h=2)

    with tile.TileContext(nc) as tc:
        with (
            tc.tile_pool(name="const", bufs=1) as cpool,
            tc.tile_pool(name="s", bufs=6) as spool,
            tc.tile_pool(name="r", bufs=4) as rpool,
            tc.tile_pool(name="slab", bufs=4) as slabpool,
        ):
            sens_bc = cpool.tile([P, W], F32)
            nc.sync.dma_start(sens_bc[:], sens_d[:, :])

            act_bias, act_scaled_bias = {}, {}
            for t in sorted(ACT_SIGN_SET):
                bt = cpool.tile([P, 1], F32, tag=f"bias{t}")
                nc.gpsimd.memset(bt[:], float(-PHIS[t]))
                act_bias[t] = bt
            for t in sorted(DIRTY_UP_SET | DIRTY_DOWN_SET):
                sc = SCALE_HI if t in DIRTY_UP_SET else SCALE_LO
                bt = cpool.tile([P, 1], F32, tag=f"biash{t}")
                nc.gpsimd.memset(bt[:], float(np.float32(-PHIS[t])
                                              * np.float32(sc)))
                act_scaled_bias[t] = bt

            xt = cpool.tile([P, W], F32)
            nc.sync.dma_start(xt[:], x_v[:, :])
            cur = cpool.tile([P, W], F32)
            nc.vector.tensor_tensor(cur[:], xt[:], sens_bc[:],
                                    mybir.AluOpType.mult)

            s_prev, prev_dirty = None, False
            for tchunk in range(TS // TC):
                slab = slabpool.tile([P, TC * W], BF16, tag="slab")
                for tt in range(TC):
                    t = tchunk * TC + tt
                    dst = slab[:, tt * W:(tt + 1) * W]
                    if t > 0 and THETAS[t] == THETAS[t - 1]:
                        # empty band: s_{t+1} == s_t, slab is identically 0
                        nc.gpsimd.memset(dst, 0.0)
                        continue
                    s = spool.tile([P, W], BF16, tag="s")
                    if t in DIRTY_UP_SET or t in DIRTY_DOWN_SET:
                        # r' = Relu(2^+-60*(cur - phi)): 0 iff cur < THETA[t]
                        sc = SCALE_HI if t in DIRTY_UP_SET else SCALE_LO
                        nc.scalar.activation(
                            s[:], cur[:], mybir.ActivationFunctionType.Relu,
                            bias=act_scaled_bias[t][:], scale=float(sc),
                        )
                    elif t in ACT_SIGN_SET:
                        r = rpool.tile([P, W], BF16, tag="r")
                        nc.scalar.activation(
                            r[:], cur[:], mybir.ActivationFunctionType.Relu,
                            bias=act_bias[t][:], scale=1.0,
                        )
                        nc.scalar.activation(
                            s[:], r[:], mybir.ActivationFunctionType.Sign,
                        )
                    else:
                        nc.vector.tensor_scalar(
                            s[:], cur[:], float(THETAS[t]), None,
                            mybir.AluOpType.is_ge,
                        )
                    if t == 0:
                        if t in DIRTY_UP_SET:
                            # slab0 = [r'_0 > 0]
                            nc.vector.tensor_scalar(
                                dst, s[:], 0.0, None, mybir.AluOpType.is_gt)
                        else:
                            nc.vector.tensor_copy(dst, s[:])
                    elif prev_dirty:
                        # prev is up-scaled: out = [r'_{t-1} < s_t]
                        nc.vector.tensor_tensor(dst, s_prev[:], s[:],
                                                mybir.AluOpType.is_lt)
                    elif t in DIRTY_DOWN_SET:
                        # cur is down-scaled: out = [s_{t-1} < r''_t]
                        nc.vector.tensor_tensor(dst, s_prev[:], s[:],
                                                mybir.AluOpType.is_lt)
                    else:
                        nc.vector.tensor_tensor(dst, s[:], s_prev[:],
                                                mybir.AluOpType.subtract)
                    s_prev, prev_dirty = s, t in DIRTY_UP_SET
                for h in range(2):
                    src = slab[:].rearrange("p (t h n) -> p t h n",
                                            t=TC, h=2, n=N)[:, :, h, :]
                    nc.sync.dma_start(
                        out_d[h * P:(h + 1) * P,
                              tchunk * TC:(tchunk + 1) * TC, :],
                        src,
                    )
    nc.compile()
    return nc


_NC = None


def _get_nc():
    global _NC
    if _NC is None:
        _NC = _build()
    return _NC


def _in_maps(x, sens):
    return [
        {"x": x[c * BS:(c + 1) * BS], "sens": sens} for c in range(N_CORES)
    ]


def kernel(x, sensitivity):
    x = np.ascontiguousarray(np.asarray(x, dtype=np.float32))
    sens1 = np.asarray(sensitivity, dtype=np.float32).reshape(1, N)
    sens = np.ascontiguousarray(np.tile(sens1, (P, 2)))   # [P, W] replicated
    nc = _get_nc()
    in_maps = _in_maps(x, sens)
    res = run_bass_kernel_spmd(nc, in_maps, list(range(N_CORES)))
    dev = np.concatenate(
        [np.asarray(r["out"]) for r in res.results], axis=0
    )  # [B, TS, N] fp8, exact 0/1
    out = np.zeros((B, T, N), dtype=np.float32)
    out[:, :TS, :] = dev.astype(np.float32)
    return out
